# revision 1
# baseline (speedup 1.0000x reference)
"""v3: GNN kernel with one-hot-matmul PSUM scatters (no dma_scatter_add).

Same math/sharding as kernel.py (v1), but segment sums are computed on the
TensorEngine: edges are block-sorted by target node; for each 128-node block
a PSUM tile accumulates  onehot^T @ [h_hi | h_lo | w_hi | w_lo]  over the
block's edge tiles, then is written to DRAM once. The one-hot (exact in
bf16) is built per edge-tile with a single fused DVE tensor_scalar
(is_equal [+ mult w]); gathered f32 rows are split hi/lo into two bf16
halves so the bf16 matmul is exact to ~2^-17.
"""
import hashlib
import numpy as np

import concourse.bacc as bacc
import concourse.bass as bass
import concourse.bass_isa as bass_isa
import concourse.mybir as mybir
import concourse.tile as tile
from concourse.bass_utils import run_bass_kernel_spmd

F32 = mybir.dt.float32
BF16 = mybir.dt.bfloat16
I16 = mybir.dt.int16
AX = mybir.AxisListType
ALU = mybir.AluOpType
ACT_EXP = mybir.ActivationFunctionType.Exp

NC = 8
C = 128
N_NODES = 50000
PER = N_NODES // NC
NTILE = (PER + 127) // 128
PAD_N = NTILE * 128
LAST_ROWS = PER - (NTILE - 1) * 128
SPLIT = 32640
CHUNK = 2048
TBLW = 136           # table row width (128 data + 1 aux + pad, 32B-aligned)


def _pad128(x):
    return (x + 127) // 128 * 128


def _block_plan(per_core_tgts, per_core_lo):
    """Per-(stream, block) padded edge counts, shared across cores."""
    plans = []
    for want_lo in (True, False):
        counts = np.zeros((NC, NTILE), np.int64)
        for r in range(NC):
            sel = per_core_lo[r] if want_lo else ~per_core_lo[r]
            t = per_core_tgts[r][sel]
            counts[r] = np.bincount(t // 128, minlength=NTILE)
        mx = counts.max(axis=0)
        padded = [(int(_pad128(m)) if m > 0 else 0) for m in mx]
        plans.append(padded)
    # ensure every block owns >= 1 tile somewhere (so its table rows get
    # written): force lo stream to 128 if both are 0
    for b in range(NTILE):
        if plans[0][b] == 0 and plans[1][b] == 0:
            plans[0][b] = 128
    return plans  # [lo_blocks, hi_blocks]


def _streams_v3(gather_idx, tgt_local, lo_mask, blocks_lo, blocks_hi):
    """Emit per-stream (g16, inblock_f32, d16) arrays ordered by block,
    padded per the shared block plan."""
    out = []
    for sel, rebase, blocks in ((lo_mask, 0, blocks_lo), (~lo_mask, SPLIT, blocks_hi)):
        g = gather_idx[sel] - rebase
        t = tgt_local[sel]
        order = np.argsort(t // 128, kind="stable")
        g, t = g[order], t[order]
        blk = t // 128
        gs, ibs, ds = [], [], []
        pos = 0
        for b, pb in enumerate(blocks):
            nb = int(np.searchsorted(blk, b + 1)) - pos if len(blk) else 0
            gs.append(g[pos:pos + nb])
            ibs.append((t[pos:pos + nb] % 128).astype(np.float32))
            ds.append(t[pos:pos + nb])
            pad = pb - nb
            gs.append(np.zeros(pad, np.int64))
            ibs.append(np.full(pad, -1.0, np.float32))
            ds.append(np.zeros(pad, np.int64))
            pos += nb
        g16 = np.concatenate(gs).astype(np.int16)
        ib = np.concatenate(ibs).astype(np.float32)
        d16 = np.concatenate(ds).astype(np.int16)
        out.append((g16, ib, d16))
    return out


def _wrap16(idx16):
    w = np.ascontiguousarray(idx16.reshape(-1, 16).T)
    return np.tile(w, (8, 1))


def _tokens_f32(arr):
    """[N] -> [128, N/128] with value of token i at [i%128, i//128]."""
    return np.ascontiguousarray(arr.reshape(-1, 128).T)


class _Plan:
    pass


def _build_program(plan, stop_after="full"):
    nc = bacc.Bacc("TRN2", target_bir_lowering=False)

    t1lo, t1hi = plan["t1"]
    t2lo, t2hi = plan["t2"]
    t2tot = t2lo + t2hi
    LCOLS = t2tot // 128

    x_src = nc.dram_tensor("x_src", [N_NODES, C], F32, kind="ExternalInput")
    x_mid = nc.dram_tensor("x_mid", [PAD_N, C], F32, kind="ExternalInput")
    x_dst = nc.dram_tensor("x_dst", [PAD_N, C], F32, kind="ExternalInput")
    w1t = nc.dram_tensor("w1t", [C, C], F32, kind="ExternalInput")
    w2t = nc.dram_tensor("w2t", [C, C], F32, kind="ExternalInput")
    qwt = nc.dram_tensor("qwt", [C, C], F32, kind="ExternalInput")
    kw_s = nc.dram_tensor("kw_s", [C, C], F32, kind="ExternalInput")
    vwt = nc.dram_tensor("vwt", [C, C], F32, kind="ExternalInput")
    b12 = nc.dram_tensor("b12", [1, C], F32, kind="ExternalInput")
    qb = nc.dram_tensor("qb", [1, C], F32, kind="ExternalInput")
    kb_s = nc.dram_tensor("kb_s", [C, 1], F32, kind="ExternalInput")
    vb = nc.dram_tensor("vb", [1, C], F32, kind="ExternalInput")
    ident_d = nc.dram_tensor("ident", [C, C], F32, kind="ExternalInput")
    iota_d = nc.dram_tensor("iota", [128, 128], F32, kind="ExternalInput")

    e_g, e_b, e_d = {}, {}, {}
    for ph, s, tot in (("1", "lo", t1lo), ("1", "hi", t1hi),
                       ("2", "lo", t2lo), ("2", "hi", t2hi)):
        if not tot:
            continue
        e_g[ph, s] = nc.dram_tensor(f"e{ph}g_{s}", [128, tot // 16], I16,
                                    kind="ExternalInput")
        e_b[ph, s] = nc.dram_tensor(f"e{ph}b_{s}", [128, tot // 128], F32,
                                    kind="ExternalInput")
        if ph == "2":
            e_d[ph, s] = nc.dram_tensor(f"e{ph}d_{s}", [128, tot // 16], I16,
                                        kind="ExternalInput")
    out_ext = nc.dram_tensor("out", [PER, C], F32, kind="ExternalOutput")

    Sd = nc.dram_tensor("Sd", [PAD_N, TBLW], F32)
    Td = nc.dram_tensor("Td", [PAD_N, TBLW], F32)
    h_r = nc.dram_tensor("h_r", [PER, C], F32)
    h_full = nc.dram_tensor("h_full", [N_NODES, C], F32, addr_space="Shared")
    Qp = nc.dram_tensor("Qp", [PAD_N, C], F32)
    mx_in = nc.dram_tensor("mx_in", [8], F32)
    mx_out = nc.dram_tensor("mx_out", [8], F32, addr_space="Shared")
    z_in = nc.dram_tensor("z_in", [8], F32)
    z_out = nc.dram_tensor("z_out", [8], F32, addr_space="Shared")

    core_ids = list(range(NC))
    TCH = CHUNK // 128    # tiles per chunk

    with tile.TileContext(nc) as tc:
        with (
            tc.tile_pool(name="big", bufs=2) as bpool,
            tc.tile_pool(name="sm", bufs=6) as spool,
            tc.tile_pool(name="nodes", bufs=4) as npool,
            tc.tile_pool(name="psA", bufs=1, space="PSUM") as ppoolA,
            tc.tile_pool(name="psB", bufs=2, space="PSUM") as ppoolB,
            tc.tile_pool(name="psS", bufs=3, space="PSUM") as ppoolS,
        ):
            # ---- constants ----
            w1t_s = nc.alloc_sbuf_tensor("sb_w1t", [C, C], F32)
            w2t_s = nc.alloc_sbuf_tensor("sb_w2t", [C, C], F32)
            qwt_s = nc.alloc_sbuf_tensor("sb_qwt", [C, C], F32)
            kw_ss = nc.alloc_sbuf_tensor("sb_kws", [C, C], F32)
            vwt_s = nc.alloc_sbuf_tensor("sb_vwt", [C, C], F32)
            b12_s = nc.alloc_sbuf_tensor("sb_b12", [1, C], F32)
            qb_s = nc.alloc_sbuf_tensor("sb_qb", [1, C], F32)
            kb_ss = nc.alloc_sbuf_tensor("sb_kbs", [C, 1], F32)
            vb_s = nc.alloc_sbuf_tensor("sb_vb", [1, C], F32)
            ident = nc.alloc_sbuf_tensor("sb_ident", [C, C], F32)
            iota_sq = nc.alloc_sbuf_tensor("sb_iota", [128, 128], F32)
            for t, d in ((w1t_s, w1t), (w2t_s, w2t), (qwt_s, qwt), (kw_ss, kw_s),
                         (vwt_s, vwt), (b12_s, b12), (qb_s, qb), (kb_ss, kb_s),
                         (vb_s, vb), (ident, ident_d), (iota_sq, iota_d)):
                nc.sync.dma_start(t[:], d[:])
            ones_row = nc.alloc_sbuf_tensor("sb_ones_row", [1, C], F32)
            nc.vector.memset(ones_row[:], 1.0)

            # ---- index / in-block arrays ----
            idx_sb, inb_sb, d_sb = {}, {}, {}
            for (ph, s), d in e_g.items():
                t = nc.alloc_sbuf_tensor(f"sb_{d.name}", list(d.shape), I16)
                nc.sync.dma_start(t[:], d[:])
                idx_sb[ph, s] = t
            for (ph, s), d in e_b.items():
                t = nc.alloc_sbuf_tensor(f"sb_{d.name}", list(d.shape), F32)
                nc.sync.dma_start(t[:], d[:])
                inb_sb[ph, s] = t
            for (ph, s), d in e_d.items():
                t = nc.alloc_sbuf_tensor(f"sb_{d.name}", list(d.shape), I16)
                nc.sync.dma_start(t[:], d[:])
                d_sb[ph, s] = t

            # ------------------------------------------------------------------
            # streaming scatter phase: gathers + hi/lo split + block matmuls
            # ------------------------------------------------------------------
            def scatter_phase(ph, table, src_lo, src_hi, blocks, w_sb=None):
                """ph: "1" or "2". table: Sd/Td. blocks: plan["b"+ph].
                w_sb: [128, LCOLS] weights (phase 2) or None (phase 1).
                Accumulates per block: cols 0:128 hi-part, 128:256 lo-part,
                256 (+257): aux (deg ones, or w_hi/w_lo sums)."""
                totals = plan["t" + ph]
                views = {"lo": src_lo, "hi": src_hi}
                woff = {"lo": 0, "hi": totals[0]}
                # lazy per-chunk issue: gather + hi/lo split emitted just
                # before the block loop first consumes a tile of that chunk,
                # so per-engine FIFO order matches consumption order.
                slots = {}
                issued = {"lo": 0, "hi": 0}   # tiles issued per stream
                pos_st = {"lo": 0, "hi": 0}

                def issue_chunk(s):
                    si = 0 if s == "lo" else 1
                    tot = totals[si]
                    pos = pos_st[s]
                    n = min(CHUNK, tot - pos)
                    ncol = n // 128
                    g = bpool.tile([128, TCH, C], F32, tag=f"gf32_{s}")
                    nc.gpsimd.dma_gather(
                        g[:, :ncol, :], views[s],
                        idx_sb[ph, s][:, pos // 16:(pos + n) // 16],
                        n, n, C, single_packet=False)
                    ghl = bpool.tile([128, TCH, 260], BF16, tag=f"ghl_{s}")
                    nc.vector.memset(ghl[:, :ncol, 256:260], 0.0)
                    if w_sb is not None:
                        wc = (woff[s] + pos) // 128
                        wv = w_sb[:, wc:wc + ncol]
                        w3 = bass.AP(wv.tensor, wv.offset, wv.ap + [[0, C]])
                        nc.vector.tensor_mul(g[:, :ncol, :], g[:, :ncol, :], w3)
                    nc.vector.tensor_copy(ghl[:, :ncol, 0:128], g[:, :ncol, :])
                    nc.vector.tensor_sub(g[:, :ncol, :], g[:, :ncol, :],
                                         ghl[:, :ncol, 0:128])
                    nc.vector.tensor_copy(ghl[:, :ncol, 128:256], g[:, :ncol, :])
                    if w_sb is None:
                        nc.vector.memset(ghl[:, :ncol, 256:257], 1.0)
                    else:
                        wv1 = bass.AP(wv.tensor, wv.offset, wv.ap + [[0, 1]])
                        nc.vector.tensor_copy(ghl[:, :ncol, 256:257], wv1)
                        wlo = spool.tile([128, TCH], F32, tag="wlo")
                        nc.vector.tensor_sub(wlo[:, :ncol], wv,
                                             ghl[:, :ncol, 256])
                        nc.vector.tensor_copy(
                            ghl[:, :ncol, 257:258],
                            bass.AP(wlo.tensor, wlo.offset,
                                    wlo[:, :ncol].ap + [[0, 1]]))
                    for tt in range(ncol):
                        slots[s, pos // 128 + tt] = (ghl, tt)
                    issued[s] += ncol
                    pos_st[s] = pos + n

                # block loop
                tile_cursor = {"lo": 0, "hi": 0}
                for b in range(NTILE):
                    ntl = blocks[0][b] // 128
                    nth = blocks[1][b] // 128
                    if ntl + nth == 0:
                        continue
                    ps = ppoolS.tile([128, 260], F32, tag="blk")
                    first = True
                    cnt = 0
                    for s, ntx in (("lo", ntl), ("hi", nth)):
                        for _ in range(ntx):
                            while tile_cursor[s] >= issued[s]:
                                issue_chunk(s)
                            gt_tile, slot = slots[s, tile_cursor[s]]
                            gpos = tile_cursor[s]
                            oh = spool.tile([128, 128], BF16, tag="oh")
                            nc.vector.tensor_scalar(
                                oh[:], iota_sq[:],
                                inb_sb[ph, s][:, gpos:gpos + 1], None,
                                ALU.is_equal)
                            cnt += 1
                            nc.tensor.matmul(
                                ps[:], oh[:], gt_tile[:, slot, :],
                                start=first, stop=(cnt == ntl + nth))
                            first = False
                            tile_cursor[s] += 1
                    sbout = npool.tile([128, TBLW], F32, tag="sbout")
                    nc.vector.tensor_copy(sbout[:, 0:129], ps[:, 0:129])
                    nc.vector.tensor_add(sbout[:, 0:128], sbout[:, 0:128],
                                         ps[:, 128:256])
                    nc.vector.tensor_copy(sbout[:, 129:130], ps[:, 256:257])
                    nc.vector.tensor_add(sbout[:, 128:129], sbout[:, 129:130],
                                         ps[:, 257:258])
                    nc.sync.dma_start(table[b * 128:(b + 1) * 128, 0:129],
                                      sbout[:, 0:129])

            # ================= PHASE 1 =================
            scatter_phase("1", Sd, x_src[0:SPLIT, :], x_src[SPLIT:N_NODES, :],
                          plan["b1"])

            # ---- node compute: h_r, Q', beta ----
            beta_sb = nc.alloc_sbuf_tensor("sb_beta", [128, NTILE], F32)
            for t in range(NTILE):
                r0 = t * 128
                Sl = npool.tile([128, TBLW], F32, tag="ld_a")
                Xm = npool.tile([128, C], F32, tag="ld_b")
                nc.sync.dma_start(Sl[:, 0:129], Sd[r0:r0 + 128, 0:129])
                nc.sync.dma_start(Xm[:], x_mid[r0:r0 + 128, :])
                nc.vector.tensor_scalar_mul(Xm[:], Xm[:], Sl[:, 128:129])
                pT = ppoolA.tile([128, C], F32, tag="pT")
                StT = npool.tile([128, C], F32, tag="tr_a")
                nc.tensor.transpose(pT[:], Sl[:, 0:128], ident[:])
                nc.vector.tensor_copy(StT[:], pT[:])
                pT2 = ppoolA.tile([128, C], F32, tag="pT")
                XmT = npool.tile([128, C], F32, tag="tr_b")
                nc.tensor.transpose(pT2[:], Xm[:], ident[:])
                nc.vector.tensor_copy(XmT[:], pT2[:])
                pD = ppoolA.tile([1, C], F32, tag="pD")
                nc.tensor.transpose(pD[:], Sl[:, 128:129], ident[:])
                degrow = spool.tile([1, C], F32, tag="degrow")
                nc.vector.tensor_copy(degrow[:], pD[:])
                pH = ppoolB.tile([128, C], F32, tag="pH")
                nc.tensor.matmul(pH[:], StT[:], w2t_s[:], start=True, stop=False)
                nc.tensor.matmul(pH[:], XmT[:], w1t_s[:], start=False, stop=False)
                nc.tensor.matmul(pH[:], degrow[:], b12_s[:], start=False, stop=True)
                hsb = npool.tile([128, C], F32, tag="hsb")
                nc.vector.tensor_copy(hsb[:], pH[:])
                rows = 128 if t < NTILE - 1 else LAST_ROWS
                nc.sync.dma_start(h_r[r0:r0 + rows, :], hsb[:rows, :])

                Xd = npool.tile([128, C], F32, tag="ld_b")
                nc.sync.dma_start(Xd[:], x_dst[r0:r0 + 128, :])
                pT3 = ppoolA.tile([128, C], F32, tag="pT")
                XdT = npool.tile([128, C], F32, tag="tr_a")
                nc.tensor.transpose(pT3[:], Xd[:], ident[:])
                nc.vector.tensor_copy(XdT[:], pT3[:])
                pQ = ppoolB.tile([128, C], F32, tag="pH")
                nc.tensor.matmul(pQ[:], XdT[:], qwt_s[:], start=True, stop=False)
                nc.tensor.matmul(pQ[:], ones_row[:], qb_s[:], start=False, stop=True)
                Qnc = npool.tile([128, C], F32, tag="hsb")
                nc.vector.tensor_copy(Qnc[:], pQ[:])
                pT4 = ppoolA.tile([128, C], F32, tag="pT")
                QT = npool.tile([128, C], F32, tag="tr_b")
                nc.tensor.transpose(pT4[:], Qnc[:], ident[:])
                nc.vector.tensor_copy(QT[:], pT4[:])
                pQ2 = ppoolB.tile([128, C], F32, tag="pH")
                nc.tensor.matmul(pQ2[:], QT[:], kw_ss[:], start=True, stop=True)
                qpsb = npool.tile([128, C], F32, tag="qpsb")
                nc.vector.tensor_copy(qpsb[:], pQ2[:])
                nc.sync.dma_start(Qp[r0:r0 + 128, :], qpsb[:])
                pB = ppoolA.tile([128, 1], F32, tag="pB")
                nc.tensor.matmul(pB[:], QT[:], kb_ss[:], start=True, stop=True)
                nc.vector.tensor_copy(beta_sb[:, t:t + 1], pB[:])

            if stop_after == "p1":
                osb0 = npool.tile([128, C], F32, tag="hsb")
                nc.vector.memset(osb0[:], 0.0)
                for t in range(NTILE):
                    r0 = t * 128
                    rows = 128 if t < NTILE - 1 else LAST_ROWS
                    nc.sync.dma_start(out_ext[r0:r0 + rows, :], osb0[:rows, :])
            if stop_after != "p1":
                mb_l0 = nc.alloc_sbuf_tensor("sb_mb_l0", [128, 1], F32)
                nc.vector.reduce_max(mb_l0[:], beta_sb[:], axis=AX.X)
                mb_loc = nc.alloc_sbuf_tensor("sb_mb_loc", [128, 1], F32)
                nc.gpsimd.partition_all_reduce(mb_loc[:], mb_l0[:], 128,
                                               bass_isa.ReduceOp.max)

                nc.gpsimd.collective_compute(
                    "AllGather", ALU.bypass, ins=[h_r[:]], outs=[h_full[:]],
                    replica_groups=[core_ids])

                # ================= PHASE 2a: logits =================
                l_sb = nc.alloc_sbuf_tensor("sb_l", [128, LCOLS], F32)
                stream_off = {"lo": 0, "hi": t2lo}
                for s, base in (("lo", 0), ("hi", SPLIT)):
                    if ("2", s) not in idx_sb:
                        continue
                    h_view = h_full[base:min(base + SPLIT, N_NODES), :]
                    tot = t2lo if s == "lo" else t2hi
                    pos = 0
                    while pos < tot:
                        n = min(CHUNK, tot - pos)
                        ncol = n // 128
                        g = bpool.tile([128, TCH, C], F32, tag="gf32")
                        q = bpool.tile([128, TCH, C], F32, tag="qtile")
                        nc.gpsimd.dma_gather(
                            g[:, :ncol, :], h_view,
                            idx_sb["2", s][:, pos // 16:(pos + n) // 16],
                            n, n, C, single_packet=False)
                        nc.gpsimd.dma_gather(
                            q[:, :ncol, :], Qp[:],
                            d_sb["2", s][:, pos // 16:(pos + n) // 16],
                            n, n, C, single_packet=False)
                        nc.vector.tensor_mul(g[:, :ncol, :], g[:, :ncol, :],
                                             q[:, :ncol, :])
                        lcol = (stream_off[s] + pos) // 128
                        nc.vector.reduce_sum(l_sb[:, lcol:lcol + ncol],
                                             g[:, :ncol, :], axis=AX.X)
                        pos += n

                ml_l0 = nc.alloc_sbuf_tensor("sb_ml_l0", [128, 1], F32)
                nc.vector.reduce_max(ml_l0[:], l_sb[:], axis=AX.X)
                ml_loc = nc.alloc_sbuf_tensor("sb_ml_loc", [128, 1], F32)
                nc.gpsimd.partition_all_reduce(ml_loc[:], ml_l0[:], 128,
                                               bass_isa.ReduceOp.max)
                mx_sb = nc.alloc_sbuf_tensor("sb_mx", [1, 8], F32)
                nc.vector.memset(mx_sb[:], -3.0e38)
                nc.vector.tensor_copy(mx_sb[:, 0:1], ml_loc[:1, :])
                nc.vector.tensor_copy(mx_sb[:, 1:2], mb_loc[:1, :])
                nc.sync.dma_start(mx_in[:], mx_sb[:1, :])
                nc.gpsimd.collective_compute(
                    "AllReduce", ALU.max, ins=[mx_in[:]], outs=[mx_out[:]],
                    replica_groups=[core_ids])
                mxg = nc.alloc_sbuf_tensor("sb_mxg", [1, 8], F32)
                nc.sync.dma_start(mxg[:], mx_out[:].rearrange("(o f) -> o f", o=1))
                pBC = ppoolA.tile([128, 2], F32, tag="pB")
                nc.tensor.matmul(pBC[:], ones_row[:], mxg[:, 0:2], start=True,
                                 stop=True)
                m_bc = nc.alloc_sbuf_tensor("sb_mbc", [128, 2], F32)
                nc.vector.tensor_copy(m_bc[:], pBC[:])
                nm_bc = nc.alloc_sbuf_tensor("sb_nmbc", [128, 2], F32)
                nc.vector.tensor_scalar_mul(nm_bc[:], m_bc[:], -1.0)

                if stop_after == "p2a":
                    osb0 = npool.tile([128, C], F32, tag="hsb")
                    nc.vector.memset(osb0[:], 0.0)
                    for t in range(NTILE):
                        r0 = t * 128
                        rows = 128 if t < NTILE - 1 else LAST_ROWS
                        nc.sync.dma_start(out_ext[r0:r0 + rows, :], osb0[:rows, :])
                if stop_after == "full":
                    nc.scalar.activation(l_sb[:], l_sb[:], ACT_EXP,
                                         bias=nm_bc[:, 0:1], scale=1.0)

                    # ============ PHASE 2c: one-hot scatter of T', S'' =======
                    scatter_phase("2", Td, h_full[0:SPLIT, :],
                                  h_full[SPLIT:N_NODES, :], plan["b2"],
                                  w_sb=l_sb)

                    # ============ PHASE 2d: combine ============
                    exb_sb = nc.alloc_sbuf_tensor("sb_exb", [128, NTILE], F32)
                    nc.scalar.activation(exb_sb[:], beta_sb[:], ACT_EXP,
                                         bias=nm_bc[:, 1:2], scale=1.0)
                    sp_sb = nc.alloc_sbuf_tensor("sb_sp", [128, NTILE], F32)
                    for t in range(NTILE):
                        spc = npool.tile([128, TBLW], F32, tag="ld_a")
                        nc.sync.dma_start(spc[:, 0:129], Td[t * 128:(t + 1) * 128, 0:129])
                        nc.vector.tensor_copy(sp_sb[:, t:t + 1], spc[:, 128:129])
                    za = nc.alloc_sbuf_tensor("sb_za", [128, NTILE], F32)
                    nc.vector.tensor_mul(za[:], sp_sb[:], exb_sb[:])
                    zr0 = nc.alloc_sbuf_tensor("sb_zr0", [128, 1], F32)
                    nc.vector.reduce_sum(zr0[:], za[:], axis=AX.X)
                    zred = nc.alloc_sbuf_tensor("sb_zred", [128, 1], F32)
                    nc.gpsimd.partition_all_reduce(zred[:], zr0[:], 128,
                                                   bass_isa.ReduceOp.add)
                    zv = nc.alloc_sbuf_tensor("sb_zv", [1, 8], F32)
                    nc.vector.memset(zv[:], 0.0)
                    nc.vector.tensor_copy(zv[:, 0:1], zred[:1, :])
                    nc.sync.dma_start(z_in[:], zv[:1, :])
                    nc.gpsimd.collective_compute(
                        "AllReduce", ALU.add, ins=[z_in[:]], outs=[z_out[:]],
                        replica_groups=[core_ids])
                    zg = nc.alloc_sbuf_tensor("sb_zg", [1, 8], F32)
                    nc.sync.dma_start(zg[:], z_out[:].rearrange("(o f) -> o f", o=1))
                    pZ = ppoolA.tile([128, 1], F32, tag="pB")
                    nc.tensor.matmul(pZ[:], ones_row[:], zg[:, 0:1], start=True,
                                     stop=True)
                    z_bc = nc.alloc_sbuf_tensor("sb_zbc", [128, 1], F32)
                    nc.vector.tensor_copy(z_bc[:], pZ[:])
                    inv_z = nc.alloc_sbuf_tensor("sb_invz", [128, 1], F32)
                    nc.vector.reciprocal(inv_z[:], z_bc[:])

                    for t in range(NTILE):
                        r0 = t * 128
                        Tt = npool.tile([128, TBLW], F32, tag="ld_a")
                        nc.sync.dma_start(Tt[:, 0:129], Td[r0:r0 + 128, 0:129])
                        pT = ppoolA.tile([128, C], F32, tag="pT")
                        TtT = npool.tile([128, C], F32, tag="tr_a")
                        nc.tensor.transpose(pT[:], Tt[:, 0:128], ident[:])
                        nc.vector.tensor_copy(TtT[:], pT[:])
                        pS = ppoolA.tile([1, C], F32, tag="pD")
                        nc.tensor.transpose(pS[:], sp_sb[:, t:t + 1], ident[:])
                        sprow = spool.tile([1, C], F32, tag="degrow")
                        nc.vector.tensor_copy(sprow[:], pS[:])
                        pO = ppoolB.tile([128, C], F32, tag="pH")
                        nc.tensor.matmul(pO[:], TtT[:], vwt_s[:], start=True,
                                         stop=False)
                        nc.tensor.matmul(pO[:], sprow[:], vb_s[:], start=False,
                                         stop=True)
                        osb = npool.tile([128, C], F32, tag="hsb")
                        nc.vector.tensor_copy(osb[:], pO[:])
                        scale = spool.tile([128, 1], F32, tag="scale")
                        nc.vector.tensor_mul(scale[:], exb_sb[:, t:t + 1], inv_z[:])
                        nc.vector.tensor_scalar_mul(osb[:], osb[:], scale[:])
                        rows = 128 if t < NTILE - 1 else LAST_ROWS
                        nc.sync.dma_start(out_ext[r0:r0 + rows, :], osb[:rows, :])

    nc.compile()
    return nc


def _build_empty_like(plan=None):
    if plan is None:
        plan = _LAST_PLAN
    nc = bacc.Bacc("TRN2", target_bir_lowering=False)
    nc.dram_tensor("x_src", [N_NODES, C], F32, kind="ExternalInput")
    x_mid = nc.dram_tensor("x_mid", [PAD_N, C], F32, kind="ExternalInput")
    nc.dram_tensor("x_dst", [PAD_N, C], F32, kind="ExternalInput")
    for nm in ("w1t", "w2t", "qwt", "kw_s", "vwt", "ident", "iota"):
        nc.dram_tensor(nm, [C, C], F32, kind="ExternalInput")
    for nm in ("b12", "qb", "vb"):
        nc.dram_tensor(nm, [1, C], F32, kind="ExternalInput")
    nc.dram_tensor("kb_s", [C, 1], F32, kind="ExternalInput")
    for ph in ("1", "2"):
        for si, s in enumerate(("lo", "hi")):
            tot = plan["t" + ph][si]
            if not tot:
                continue
            nc.dram_tensor(f"e{ph}g_{s}", [128, tot // 16], I16, kind="ExternalInput")
            nc.dram_tensor(f"e{ph}b_{s}", [128, tot // 128], F32, kind="ExternalInput")
            if ph == "2":
                nc.dram_tensor(f"e{ph}d_{s}", [128, tot // 16], I16, kind="ExternalInput")
    out_ext = nc.dram_tensor("out", [PER, C], F32, kind="ExternalOutput")
    with tile.TileContext(nc) as tc:
        with tc.tile_pool(name="p", bufs=1) as pool:
            t = pool.tile([128, C], F32)
            nc.sync.dma_start(t[:], x_mid[0:128, :])
            nc.sync.dma_start(out_ext[0:128, :], t[:])
    nc.compile()
    return nc


_CACHE = {}
_LAST_NC = None
_LAST_INMAPS = None
_LAST_PLAN = None


def kernel(x_src, x_mid, x_dst, edge_index_1, edge_index_2,
           W1_w, W1_b, W2_w, W2_b, q_w, q_b, k_w, k_b, v_w, v_b,
           stop_after="full"):
    global _LAST_NC, _LAST_INMAPS, _LAST_PLAN
    x_src = np.ascontiguousarray(np.asarray(x_src, np.float32))
    x_mid = np.ascontiguousarray(np.asarray(x_mid, np.float32))
    x_dst = np.ascontiguousarray(np.asarray(x_dst, np.float32))
    e1 = np.asarray(edge_index_1, np.int64)
    e2 = np.asarray(edge_index_2, np.int64)

    s1, m1 = e1[0], e1[1]
    m2, d2 = e2[0], e2[1]
    own1 = m1 // PER
    own2 = d2 // PER
    per1_g, per1_t, per1_lo = [], [], []
    per2_g, per2_t, per2_lo = [], [], []
    for r in range(NC):
        i = np.flatnonzero(own1 == r)
        per1_g.append(s1[i]); per1_t.append(m1[i] - r * PER)
        per1_lo.append(s1[i] < SPLIT)
        j = np.flatnonzero(own2 == r)
        per2_g.append(m2[j]); per2_t.append(d2[j] - r * PER)
        per2_lo.append(m2[j] < SPLIT)

    b1 = _block_plan(per1_t, per1_lo)
    b2 = _block_plan(per2_t, per2_lo)
    plan = {
        "b1": b1, "b2": b2,
        "t1": (sum(b1[0]), sum(b1[1])),
        "t2": (sum(b2[0]), sum(b2[1])),
    }

    key = hashlib.sha256(
        e1.tobytes() + e2.tobytes() + str(plan["t1"] + plan["t2"]).encode()
        + stop_after.encode() + b"v3"
    ).hexdigest()
    if key in _CACHE:
        nc = _CACHE[key]
    else:
        nc = _build_program(plan, stop_after=stop_after)
        _CACHE[key] = nc

    sqc = np.float32(np.sqrt(C))
    common = {
        "x_src": x_src,
        "w1t": np.ascontiguousarray(np.asarray(W1_w, np.float32).T),
        "w2t": np.ascontiguousarray(np.asarray(W2_w, np.float32).T),
        "qwt": np.ascontiguousarray(np.asarray(q_w, np.float32).T),
        "kw_s": np.ascontiguousarray(np.asarray(k_w, np.float32) / sqc),
        "vwt": np.ascontiguousarray(np.asarray(v_w, np.float32).T),
        "b12": (np.asarray(W1_b, np.float32) + np.asarray(W2_b, np.float32))[None, :],
        "qb": np.asarray(q_b, np.float32)[None, :],
        "kb_s": (np.asarray(k_b, np.float32) / sqc)[:, None].copy(),
        "vb": np.asarray(v_b, np.float32)[None, :],
        "ident": np.eye(C, dtype=np.float32),
        "iota": np.tile(np.arange(128, dtype=np.float32)[None, :], (128, 1)),
    }
    in_maps = []
    for r in range(NC):
        xm = np.zeros((PAD_N, C), np.float32)
        xm[:PER] = x_mid[r * PER:(r + 1) * PER]
        xd = np.zeros((PAD_N, C), np.float32)
        xd[:PER] = x_dst[r * PER:(r + 1) * PER]
        st1 = _streams_v3(per1_g[r], per1_t[r], per1_lo[r], b1[0], b1[1])
        st2 = _streams_v3(per2_g[r], per2_t[r], per2_lo[r], b2[0], b2[1])
        m = {"x_mid": xm, "x_dst": xd, **common}
        for sname, (g16, ib, d16) in zip(("lo", "hi"), st1):
            if len(g16):
                m[f"e1g_{sname}"] = _wrap16(g16)
                m[f"e1b_{sname}"] = _tokens_f32(ib)
        for sname, (g16, ib, d16) in zip(("lo", "hi"), st2):
            if len(g16):
                m[f"e2g_{sname}"] = _wrap16(g16)
                m[f"e2b_{sname}"] = _tokens_f32(ib)
                m[f"e2d_{sname}"] = _wrap16(d16)
        in_maps.append(m)

    _LAST_NC, _LAST_INMAPS, _LAST_PLAN = nc, in_maps, plan

    import os
    if os.environ.get("GNN_SIM"):
        from concourse.bass_interp import MultiCoreSim
        sim = MultiCoreSim(nc, NC)
        for r in range(NC):
            for name, arr in in_maps[r].items():
                sim.cores[r].tensor(name)[:] = arr
        sim.simulate()
        out = np.concatenate(
            [np.asarray(sim.cores[r].mem_tensor("out")) for r in range(NC)], axis=0)
        return out
    res = run_bass_kernel_spmd(nc, in_maps, list(range(NC)))
    out = np.concatenate([res.results[r]["out"] for r in range(NC)], axis=0)
    return out



# revision 17
# speedup vs baseline: 4.3467x; 4.3467x over previous
"""v4: GNN kernel — single-pass phase 2 (no max-subtraction in softmax),
transposed one-hot scatters, SBUF-resident tables, balanced node blocks.

Key differences vs v3:
- Softmax: logits are in [-30, 30] for any plausible input scale, so
  exp() is computed directly in f32 (softmax is shift-invariant; no
  global max all-reduce, no second logits pass). Phase 2 gathers h ONCE.
- Phase-2 Q' rows come from an SBUF-resident per-core table via a
  transposed one-hot matmul (no DMA gather of Q').
- Scatters produce TRANSPOSED tables (lhsT=data, rhs=onehot), so the
  node-side matmuls consume them directly with zero TensorE transposes.
- h is stored/all-gathered/gathered in bf16; phase-1 x_src gathered f32
  and converted to bf16 on the Scalar engine.
- Per-core node permutation (greedy degree bin-packing) balances edges
  across 128-node blocks, cutting per-block padding to ~10%.
"""
import hashlib
import heapq
import numpy as np
import ml_dtypes

import concourse.bacc as bacc
import concourse.bass as bass
import concourse.bass_isa as bass_isa
import concourse.mybir as mybir
import concourse.tile as tile
from concourse.bass_utils import run_bass_kernel_spmd

F32 = mybir.dt.float32
BF16 = mybir.dt.bfloat16
I16 = mybir.dt.int16
AX = mybir.AxisListType
ALU = mybir.AluOpType
ACT = mybir.ActivationFunctionType

NC = 8
C = 128
N_NODES = 50000
PER = N_NODES // NC          # 6250
NTILE = (PER + 127) // 128   # 49
PAD_N = NTILE * 128          # 6272
SPLIT = 32640                # int16 gather-index limit (multiple of 128)
CHUNK1 = 2048
TCH1 = CHUNK1 // 128
CHUNK2 = 4096
TCH2 = CHUNK2 // 128

BFNP = ml_dtypes.bfloat16


def _pad128(x):
    return (x + 127) // 128 * 128


def _pack_blocks(deg):
    """Greedy bin-packing of PER nodes into NTILE blocks of <=128 nodes,
    balancing total degree. Returns slot[local_node] in [0, PAD_N)."""
    order = np.argsort(-deg, kind="stable")
    heap = [(0, b) for b in range(NTILE)]
    heapq.heapify(heap)
    cnt = np.zeros(NTILE, np.int64)
    slot = np.empty(PER, np.int64)
    spill = []
    for i in order:
        while True:
            tot, b = heapq.heappop(heap)
            if cnt[b] < 128:
                break
        slot[i] = b * 128 + cnt[b]
        cnt[b] += 1
        heapq.heappush(heap, (tot + int(deg[i]), b))
    return slot


def _block_plan(per_core_t, per_core_lo):
    """Shared (max-over-cores) padded per-(block, stream) edge counts."""
    plans = []
    for want_lo in (True, False):
        counts = np.zeros((NC, NTILE), np.int64)
        for r in range(NC):
            sel = per_core_lo[r] if want_lo else ~per_core_lo[r]
            t = per_core_t[r][sel]
            counts[r] = np.bincount(t // 128, minlength=NTILE)
        mx = counts.max(axis=0)
        plans.append([(int(_pad128(m)) if m > 0 else 0) for m in mx])
    for b in range(NTILE):
        if plans[0][b] == 0 and plans[1][b] == 0:
            plans[0][b] = 128
    return plans


def _streams(g_idx, t_slot, lo_mask, blocks_lo, blocks_hi):
    """Per-stream (g16, inb_bf16) arrays ordered by block, padded."""
    out = []
    for sel, rebase, blocks in ((lo_mask, 0, blocks_lo),
                                (~lo_mask, SPLIT, blocks_hi)):
        g = g_idx[sel] - rebase
        t = t_slot[sel]
        order = np.argsort(t // 128, kind="stable")
        g, t = g[order], t[order]
        blk = t // 128
        gs, ibs = [], []
        pos = 0
        for b, pb in enumerate(blocks):
            nb = int(np.searchsorted(blk, b + 1)) - pos if len(blk) else 0
            gs.append(g[pos:pos + nb])
            ibs.append((t[pos:pos + nb] % 128).astype(np.float32))
            pad = pb - nb
            gs.append(np.zeros(pad, np.int64))
            ibs.append(np.full(pad, -1.0, np.float32))
            pos += nb
        g16 = np.concatenate(gs).astype(np.int16)
        ib = np.concatenate(ibs).astype(np.float32)
        out.append((g16, ib))
    return out


def _wrap16(idx16):
    w = np.ascontiguousarray(idx16.reshape(-1, 16).T)
    return np.tile(w, (8, 1))


def _tokens(arr):
    """[N] -> [128, N/128] with value of token i at [i%128, i//128]."""
    return np.ascontiguousarray(arr.reshape(-1, 128).T)


def _build_program(plan, stop_after="full", ttr_mode="mul"):
    TTR_MODE = ttr_mode  # noqa: N806 (read in nested scope)
    nc = bacc.Bacc("TRN2", target_bir_lowering=False)

    t1lo, t1hi = plan["t1"]
    t2lo, t2hi = plan["t2"]

    x_src = nc.dram_tensor("x_src", [N_NODES, C], F32, kind="ExternalInput")
    xmT = nc.dram_tensor("xmT", [C, PAD_N], F32, kind="ExternalInput")
    xdT = nc.dram_tensor("xdT", [C, PAD_N], F32, kind="ExternalInput")
    w1t = nc.dram_tensor("w1t", [C, C], F32, kind="ExternalInput")
    w2t = nc.dram_tensor("w2t", [C, C], F32, kind="ExternalInput")
    qwt = nc.dram_tensor("qwt", [C, C], F32, kind="ExternalInput")
    kw_s = nc.dram_tensor("kw_s", [C, C], F32, kind="ExternalInput")
    vwt = nc.dram_tensor("vwt", [C, C], F32, kind="ExternalInput")
    b12 = nc.dram_tensor("b12", [1, C], F32, kind="ExternalInput")
    qb_c = nc.dram_tensor("qb_c", [C, 1], F32, kind="ExternalInput")
    kb_ss = nc.dram_tensor("kb_s", [C, 1], F32, kind="ExternalInput")
    vb = nc.dram_tensor("vb", [1, C], F32, kind="ExternalInput")
    ident = nc.dram_tensor("ident", [C, C], F32, kind="ExternalInput")
    identb = nc.dram_tensor("identb", [C, C], BF16, kind="ExternalInput")
    iota_b = nc.dram_tensor("iota_b", [128, 128], BF16, kind="ExternalInput")
    deg_c = nc.dram_tensor("deg_c", [128, NTILE], F32, kind="ExternalInput")

    e_g, e_b = {}, {}
    for ph, s, tot in (("1", "lo", t1lo), ("1", "hi", t1hi),
                       ("2", "lo", t2lo), ("2", "hi", t2hi)):
        if not tot:
            continue
        e_g[ph, s] = nc.dram_tensor(f"e{ph}g_{s}", [128, tot // 16], I16,
                                    kind="ExternalInput")
        e_b[ph, s] = nc.dram_tensor(f"e{ph}b_{s}", [128, tot // 128], F32,
                                    kind="ExternalInput")
    out_ext = nc.dram_tensor("out", [PAD_N, C], F32, kind="ExternalOutput")

    h_r = nc.dram_tensor("h_r", [PAD_N, C], BF16)
    h_full = nc.dram_tensor("h_full", [NC * PAD_N, C], BF16,
                            addr_space="Shared")
    z_in = nc.dram_tensor("z_in", [8], F32)
    z_out = nc.dram_tensor("z_out", [8], F32, addr_space="Shared")

    core_ids = list(range(NC))

    with tile.TileContext(nc) as tc:
        with (
            tc.tile_pool(name="g1", bufs=3) as g1pool,
            tc.tile_pool(name="g1b", bufs=3) as g1bpool,
            tc.tile_pool(name="g2", bufs=3) as g2pool,
            tc.tile_pool(name="oh", bufs=4) as ohpool,
            tc.tile_pool(name="oht", bufs=3) as ohtpool,
            tc.tile_pool(name="scr", bufs=3) as scrpool,
            tc.tile_pool(name="wv", bufs=6) as wpool,
            tc.tile_pool(name="nd", bufs=4) as npool,
            tc.tile_pool(name="psmm", bufs=4, space="PSUM") as ppmm,
            tc.tile_pool(name="psacc", bufs=2, space="PSUM") as ppacc,
            tc.tile_pool(name="pss", bufs=2, space="PSUM") as pps,
        ):
            # ---------------- constants ----------------
            w1t_s = nc.alloc_sbuf_tensor("sb_w1t", [C, C], F32)
            w2t_s = nc.alloc_sbuf_tensor("sb_w2t", [C, C], F32)
            qwt_s = nc.alloc_sbuf_tensor("sb_qwt", [C, C], F32)
            kw_ss = nc.alloc_sbuf_tensor("sb_kws", [C, C], F32)
            vwt_s = nc.alloc_sbuf_tensor("sb_vwt", [C, C], F32)
            b12_s = nc.alloc_sbuf_tensor("sb_b12", [1, C], F32)
            qb_s = nc.alloc_sbuf_tensor("sb_qb", [C, 1], F32)
            kb_s = nc.alloc_sbuf_tensor("sb_kb", [C, 1], F32)
            vb_s = nc.alloc_sbuf_tensor("sb_vb", [1, C], F32)
            id_s = nc.alloc_sbuf_tensor("sb_id", [C, C], F32)
            idb_s = nc.alloc_sbuf_tensor("sb_idb", [C, C], BF16)
            iob_s = nc.alloc_sbuf_tensor("sb_iob", [128, 128], BF16)
            degc_s = nc.alloc_sbuf_tensor("sb_degc", [128, NTILE], F32)
            for t, d in ((w1t_s, w1t), (w2t_s, w2t), (qwt_s, qwt),
                         (kw_ss, kw_s), (vwt_s, vwt), (b12_s, b12),
                         (qb_s, qb_c), (kb_s, kb_ss), (vb_s, vb),
                         (id_s, ident), (idb_s, identb), (iob_s, iota_b),
                         (degc_s, deg_c)):
                nc.sync.dma_start(t[:], d[:])
            onesb = nc.alloc_sbuf_tensor("sb_onesb", [128, 1], BF16)
            nc.vector.memset(onesb[:], 1.0)

            idx_sb, inb_sb = {}, {}
            for (ph, s), d in e_g.items():
                t = nc.alloc_sbuf_tensor(f"sb_{d.name}", list(d.shape), I16)
                nc.sync.dma_start(t[:], d[:])
                idx_sb[ph, s] = t
            for (ph, s), d in e_b.items():
                t = nc.alloc_sbuf_tensor(f"sb_{d.name}", list(d.shape), F32)
                nc.sync.dma_start(t[:], d[:])
                inb_sb[ph, s] = t

            # ---------------- tables ----------------
            ST = nc.alloc_sbuf_tensor("sb_ST", [128, PAD_N], F32)
            QP = nc.alloc_sbuf_tensor("sb_QP", [128, PAD_N], BF16)
            TT = nc.alloc_sbuf_tensor("sb_TT", [128, PAD_N], F32)
            s_sb = nc.alloc_sbuf_tensor("sb_s", [1, PAD_N], F32)
            gcol = nc.alloc_sbuf_tensor("sb_gcol", [128, NTILE], F32)
            zacc = [nc.alloc_sbuf_tensor("sb_zacc0", [128, 1], F32),
                    nc.alloc_sbuf_tensor("sb_zacc1", [128, 1], F32)]
            nc.vector.memset(zacc[0][:], 0.0)

            # ============ PHASE A: Q' table, gamma ============
            for b in range(NTILE):
                cs = slice(b * 128, (b + 1) * 128)
                xd = npool.tile([128, C], F32, tag="xd")
                nc.sync.dma_start(xd[:], xdT[:, cs])
                pQ = ppmm.tile([128, C], F32, tag="mm")
                nc.tensor.matmul(pQ[:], qwt_s[:], xd[:], start=True, stop=True)
                qT = npool.tile([128, C], F32, tag="qT")
                nc.scalar.activation(qT[:], pQ[:], ACT.Identity, bias=qb_s[:])
                pQp = ppmm.tile([128, C], F32, tag="mm")
                nc.tensor.matmul(pQp[:], qT[:], kw_ss[:], start=True, stop=True)
                nc.vector.tensor_copy(QP[:, cs], pQp[:])
                pB = ppmm.tile([128, 1], F32, tag="mm")
                nc.tensor.matmul(pB[:], qT[:], kb_s[:], start=True, stop=True)
                nc.scalar.activation(gcol[:, b:b + 1], pB[:], ACT.Exp)

            _ORD = {"A": 0, "B": 1, "C": 2, "D": 3, "E1": 4, "E2": 4,
                    "E3": 4, "E": 4, "E2a": 4, "E2m": 4, "full": 5}

            def _want(ph):
                return _ORD[stop_after] >= _ORD[ph]

            def _zero_out():
                zt = npool.tile([128, C], F32, tag="ob")
                nc.vector.memset(zt[:], 0.0)
                for t in range(NTILE):
                    nc.sync.dma_start(out_ext[t * 128:(t + 1) * 128, :], zt[:])

            # ============ gather-chunk machinery ============
            def make_chunker(ph, views, totals, to_bf16):
                slots = {}
                issued = {"lo": 0, "hi": 0}
                pos_st = {"lo": 0, "hi": 0}
                chunk = CHUNK1 if to_bf16 else CHUNK2
                tch = chunk // 128

                def issue(s):
                    tot = totals[0] if s == "lo" else totals[1]
                    pos = pos_st[s]
                    n = min(chunk, tot - pos)
                    ncol = n // 128
                    if to_bf16:
                        g = g1pool.tile([128, tch, C], F32, tag="g1")
                        nc.gpsimd.dma_gather(
                            g[:, :ncol, :], views[s],
                            idx_sb[ph, s][:, pos // 16:(pos + n) // 16],
                            n, n, C, single_packet=False)
                        gb = g1bpool.tile([128, tch, C], BF16, tag="g1b")
                        nc.scalar.activation(gb[:, :ncol, :], g[:, :ncol, :],
                                             ACT.Copy)
                    else:
                        gb = g2pool.tile([128, tch, C], BF16, tag="g2")
                        nc.gpsimd.dma_gather(
                            gb[:, :ncol, :], views[s],
                            idx_sb[ph, s][:, pos // 16:(pos + n) // 16],
                            n, n, C, single_packet=False)
                    for tt in range(ncol):
                        slots[s, pos // 128 + tt] = (gb, tt)
                    issued[s] += ncol
                    pos_st[s] = pos + n

                def get_tile(s, tix):
                    while tix >= issued[s]:
                        issue(s)
                    return slots[s, tix]

                return get_tile

            # ============ PHASE B: phase-1 scatter -> ST ============
            if not _want("B"):
                _zero_out()
            get1 = make_chunker("1", {"lo": x_src[0:SPLIT, :],
                                      "hi": x_src[SPLIT:N_NODES, :]},
                                plan["t1"], to_bf16=True)
            cur1 = {"lo": 0, "hi": 0}
            items1 = []
            for b in range(NTILE):
                ntl = plan["b1"][0][b] // 128
                nth = plan["b1"][1][b] // 128
                tot = ntl + nth
                cnt = 0
                for s, ntx in (("lo", ntl), ("hi", nth)):
                    for _ in range(ntx):
                        cnt += 1
                        items1.append((b, s, cnt == 1, cnt == tot))
            st1 = {}
            acc1 = [None]

            def b_stageA(i):
                b, s, first, last = items1[i]
                gb, slot = get1(s, cur1[s])
                gpos = cur1[s]
                cur1[s] += 1
                oh = ohpool.tile([128, 128], BF16, tag="oh1")
                nc.vector.tensor_scalar(
                    oh[:], iob_s[:],
                    inb_sb["1", s][:, gpos:gpos + 1], None,
                    ALU.is_equal)
                st1[i] = (b, first, last, gb, slot, oh)

            def b_stageB(i):
                b, first, last, gb, slot, oh = st1.pop(i)
                if first:
                    acc1[0] = ppacc.tile([128, 128], F32, tag="acc", name="acc1t")
                nc.tensor.matmul(acc1[0][:], gb[:, slot, :], oh[:],
                                 start=first, stop=last)
                if last:
                    nc.vector.tensor_copy(
                        ST[:, b * 128:(b + 1) * 128], acc1[0][:])

            if _want("B"):
                LA1 = 2
                for i in range(len(items1)):
                    b_stageA(i)
                    if i >= LA1:
                        b_stageB(i - LA1)
                for i in range(len(items1) - LA1, len(items1)):
                    b_stageB(i)
            if stop_after == "B":
                _zero_out()

            # ============ PHASE C: h = deg*(x_mid W1^T + b12) + S W2^T ====
            for b in range(NTILE if _want("C") else 0):
                cs = slice(b * 128, (b + 1) * 128)
                xm = npool.tile([128, C], F32, tag="xm")
                nc.sync.dma_start(xm[:], xmT[:, cs])
                pH1 = ppmm.tile([128, C], F32, tag="mm")
                nc.tensor.matmul(pH1[:], xm[:], w1t_s[:], start=True, stop=True)
                h1 = npool.tile([128, C], F32, tag="h1")
                nc.vector.tensor_scalar_mul(h1[:], pH1[:], degc_s[:, b:b + 1])
                pDr = ppmm.tile([1, 128], F32, tag="mm")
                nc.tensor.transpose(pDr[:], degc_s[:, b:b + 1], id_s[:])
                dr = npool.tile([1, 128], F32, tag="dr")
                nc.scalar.activation(dr[:], pDr[:], ACT.Copy)
                pH2 = ppmm.tile([128, C], F32, tag="mm")
                nc.tensor.matmul(pH2[:], ST[:, cs], w2t_s[:],
                                 start=True, stop=False)
                nc.tensor.matmul(pH2[:], dr[:], b12_s[:],
                                 start=False, stop=True)
                hb = npool.tile([128, C], BF16, tag="hb")
                nc.vector.tensor_add(hb[:], h1[:], pH2[:])
                nc.sync.dma_start(h_r[b * 128:(b + 1) * 128, :], hb[:])

            if stop_after == "C":
                _zero_out()
            # ============ PHASE D: AllGather h ============
            if _want("D"):
                nc.gpsimd.collective_compute(
                    "AllGather", ALU.bypass, ins=[h_r[:]], outs=[h_full[:]],
                    replica_groups=[core_ids])
            if stop_after == "D":
                _zero_out()

            # ============ PHASE E: phase-2 single pass ============
            get2 = make_chunker("2", {"lo": h_full[0:SPLIT, :],
                                      "hi": h_full[SPLIT:NC * PAD_N, :]},
                                plan["t2"], to_bf16=False)
            cur2 = {"lo": 0, "hi": 0}
            items2 = []
            for b in range(NTILE):
                ntl = plan["b2"][0][b] // 128
                nth = plan["b2"][1][b] // 128
                tot = ntl + nth
                cnt = 0
                for s, ntx in (("lo", ntl), ("hi", nth)):
                    for _ in range(ntx):
                        cnt += 1
                        items2.append((b, s, cnt == 1, cnt == tot))
            st2 = {}
            acc2 = [None, None]

            def e_stageA(i):
                b, s, first, last = items2[i]
                cs = slice(b * 128, (b + 1) * 128)
                gb, slot = get2(s, cur2[s])
                gpos = cur2[s]
                cur2[s] += 1
                oh = ohpool.tile([128, 128], BF16, tag="oh2")
                nc.vector.tensor_scalar(
                    oh[:], iob_s[:],
                    inb_sb["2", s][:, gpos:gpos + 1], None,
                    ALU.is_equal)
                ptr = ppmm.tile([128, 128], BF16, tag="mm")
                nc.tensor.transpose(ptr[:], oh[:], idb_s[:])
                ohT = ohtpool.tile([128, 128], BF16, tag="ohT")
                nc.scalar.activation(ohT[:], ptr[:], ACT.Copy)
                qr = ppmm.tile([128, 128], F32, tag="mm")
                nc.tensor.matmul(qr[:], ohT[:], QP[:, cs],
                                 start=True, stop=True)
                st2[i] = (b, first, last, gb, slot, oh, qr)

            def e_stageB(i):
                b, first, last, gb, slot, oh, qr = st2.pop(i)
                cs = slice(b * 128, (b + 1) * 128)
                if stop_after == "E1":
                    return
                qrb = ohtpool.tile([128, 128], BF16, tag="qrb")
                nc.scalar.activation(qrb[:], qr[:], ACT.Copy)
                lw = wpool.tile([128, 1], F32, tag="lw")
                if TTR_MODE == "ttr":
                    scr = scrpool.tile([128, 128], BF16, tag="scr")
                    nc.vector.tensor_tensor_reduce(
                        scr[:], gb[:, slot, :], qrb[:], 1.0, 0.0,
                        ALU.mult, ALU.add, lw[:])
                else:
                    scr = scrpool.tile([128, 128], F32, tag="scr")
                    nc.vector.tensor_mul(scr[:], gb[:, slot, :], qrb[:])
                    nc.vector.reduce_sum(lw[:], scr[:], axis=AX.X)
                if stop_after == "E2a":
                    return
                wv = wpool.tile([128, 1], F32, tag="wv")
                nc.scalar.activation(wv[:], lw[:], ACT.Exp)
                if stop_after in ("E2", "E2m"):
                    return
                ohw = ohpool.tile([128, 128], BF16, tag="ohw")
                nc.vector.tensor_scalar_mul(ohw[:], oh[:], wv[:])
                if stop_after == "E3":
                    return
                if first:
                    acc2[0] = ppacc.tile([128, 128], F32, tag="acc", name="acc2t")
                    acc2[1] = pps.tile([1, 128], F32, tag="sS", name="acc2s")
                psT, psS = acc2
                nc.tensor.matmul(psT[:], gb[:, slot, :], ohw[:],
                                 start=first, stop=last)
                nc.tensor.matmul(psS[:], onesb[:], ohw[:],
                                 start=first, stop=last)
                if last:
                    nc.vector.tensor_copy(TT[:, cs], psT[:])
                    nc.vector.tensor_copy(s_sb[0:1, cs], psS[:])
                    pSt = ppmm.tile([128, 1], F32, tag="mm")
                    nc.tensor.transpose(pSt[:], s_sb[0:1, cs],
                                        id_s[0:1, 0:1])
                    zscr = wpool.tile([128, 1], F32, tag="zscr")
                    za, zb_ = zacc[b % 2], zacc[(b + 1) % 2]
                    nc.vector.tensor_mul(zscr[:], pSt[:], gcol[:, b:b + 1])
                    nc.vector.tensor_add(zb_[:], zscr[:], za[:])

            if _want("E1"):
                LA2 = 2
                for i in range(len(items2)):
                    e_stageA(i)
                    if i >= LA2:
                        e_stageB(i - LA2)
                for i in range(len(items2) - LA2, len(items2)):
                    e_stageB(i)
            if stop_after in ("E1", "E2", "E3", "E"):
                _zero_out()

            # ============ PHASE F: Z all-reduce ============
            if stop_after == "full":
                zfin = zacc[NTILE % 2]
                zred = nc.alloc_sbuf_tensor("sb_zred", [128, 1], F32)
                nc.gpsimd.partition_all_reduce(zred[:], zfin[:], 128,
                                               bass_isa.ReduceOp.add)
                zv = nc.alloc_sbuf_tensor("sb_zv", [1, 8], F32)
                nc.vector.memset(zv[:], 0.0)
                nc.vector.tensor_copy(zv[:, 0:1], zred[:1, :])
                nc.sync.dma_start(z_in[:], zv[:1, :])
                nc.gpsimd.collective_compute(
                    "AllReduce", ALU.add, ins=[z_in[:]], outs=[z_out[:]],
                    replica_groups=[core_ids])
                zg = nc.alloc_sbuf_tensor("sb_zg", [1, 8], F32)
                nc.sync.dma_start(zg[:],
                                  z_out[:].rearrange("(o f) -> o f", o=1))
                inv = nc.alloc_sbuf_tensor("sb_inv", [1, 1], F32)
                nc.vector.reciprocal(inv[:], zg[:, 0:1])
                invbc = nc.alloc_sbuf_tensor("sb_invbc", [128, 1], F32)
                nc.gpsimd.partition_broadcast(invbc[:], inv[:])
                scl = nc.alloc_sbuf_tensor("sb_scl", [128, NTILE], F32)
                nc.vector.tensor_scalar_mul(scl[:], gcol[:], invbc[:])

                # ============ PHASE G: output ============
                for b in range(NTILE):
                    cs = slice(b * 128, (b + 1) * 128)
                    pO = ppmm.tile([128, C], F32, tag="mm")
                    nc.tensor.matmul(pO[:], TT[:, cs], vwt_s[:],
                                     start=True, stop=False)
                    nc.tensor.matmul(pO[:], s_sb[0:1, cs], vb_s[:],
                                     start=False, stop=True)
                    ob = npool.tile([128, C], F32, tag="ob")
                    nc.vector.tensor_scalar_mul(ob[:], pO[:], scl[:, b:b + 1])
                    nc.sync.dma_start(out_ext[b * 128:(b + 1) * 128, :], ob[:])

    nc.compile()
    return nc


def _build_empty_like(plan):
    nc = bacc.Bacc("TRN2", target_bir_lowering=False)
    nc.dram_tensor("x_src", [N_NODES, C], F32, kind="ExternalInput")
    x_mT = nc.dram_tensor("xmT", [C, PAD_N], F32, kind="ExternalInput")
    nc.dram_tensor("xdT", [C, PAD_N], F32, kind="ExternalInput")
    for nm in ("w1t", "w2t", "qwt", "kw_s", "vwt"):
        nc.dram_tensor(nm, [C, C], F32, kind="ExternalInput")
    nc.dram_tensor("b12", [1, C], F32, kind="ExternalInput")
    nc.dram_tensor("qb_c", [C, 1], F32, kind="ExternalInput")
    nc.dram_tensor("kb_s", [C, 1], F32, kind="ExternalInput")
    nc.dram_tensor("vb", [1, C], F32, kind="ExternalInput")
    nc.dram_tensor("ident", [C, C], F32, kind="ExternalInput")
    nc.dram_tensor("identb", [C, C], BF16, kind="ExternalInput")
    nc.dram_tensor("iota_b", [128, 128], BF16, kind="ExternalInput")
    nc.dram_tensor("deg_c", [128, NTILE], F32, kind="ExternalInput")
    for ph, s, tot in (("1", "lo", plan["t1"][0]), ("1", "hi", plan["t1"][1]),
                       ("2", "lo", plan["t2"][0]), ("2", "hi", plan["t2"][1])):
        if not tot:
            continue
        nc.dram_tensor(f"e{ph}g_{s}", [128, tot // 16], I16,
                       kind="ExternalInput")
        nc.dram_tensor(f"e{ph}b_{s}", [128, tot // 128], F32,
                       kind="ExternalInput")
    out_ext = nc.dram_tensor("out", [PAD_N, C], F32, kind="ExternalOutput")
    with tile.TileContext(nc) as tc:
        with tc.tile_pool(name="p", bufs=1) as pool:
            t = pool.tile([128, C], F32)
            nc.sync.dma_start(t[:], x_mT[:, 0:C])
            nc.sync.dma_start(out_ext[0:128, :], t[:])
    nc.compile()
    return nc


_CACHE = {}
_LAST_NC = None
_LAST_INMAPS = None
_LAST_PLAN = None


def kernel(x_src, x_mid, x_dst, edge_index_1, edge_index_2,
           W1_w, W1_b, W2_w, W2_b, q_w, q_b, k_w, k_b, v_w, v_b,
           stop_after="full", ttr_mode="mul"):
    global _LAST_NC, _LAST_INMAPS, _LAST_PLAN
    x_src = np.ascontiguousarray(np.asarray(x_src, np.float32))
    x_mid = np.ascontiguousarray(np.asarray(x_mid, np.float32))
    x_dst = np.ascontiguousarray(np.asarray(x_dst, np.float32))
    e1 = np.asarray(edge_index_1, np.int64)
    e2 = np.asarray(edge_index_2, np.int64)

    s1, m1 = e1[0], e1[1]
    m2, d2 = e2[0], e2[1]
    own1 = m1 // PER
    own2 = d2 // PER

    # per-core balanced permutations
    perm1, perm2 = [], []          # local node -> slot
    for r in range(NC):
        lo, hi = r * PER, (r + 1) * PER
        deg1 = np.bincount(m1[(m1 >= lo) & (m1 < hi)] - lo, minlength=PER)
        deg2 = np.bincount(d2[(d2 >= lo) & (d2 < hi)] - lo, minlength=PER)
        perm1.append(_pack_blocks(deg1))
        perm2.append(_pack_blocks(deg2))

    # permuted global h-row of every mid node
    h_row = np.empty(N_NODES, np.int64)
    for r in range(NC):
        lo = r * PER
        h_row[lo:lo + PER] = r * PAD_N + perm1[r]

    per1_g, per1_t, per1_lo = [], [], []
    per2_g, per2_t, per2_lo = [], [], []
    for r in range(NC):
        i = np.flatnonzero(own1 == r)
        per1_g.append(s1[i])
        per1_t.append(perm1[r][m1[i] - r * PER])
        per1_lo.append(s1[i] < SPLIT)
        j = np.flatnonzero(own2 == r)
        g2 = h_row[m2[j]]
        per2_g.append(g2)
        per2_t.append(perm2[r][d2[j] - r * PER])
        per2_lo.append(g2 < SPLIT)

    b1 = _block_plan(per1_t, per1_lo)
    b2 = _block_plan(per2_t, per2_lo)
    plan = {
        "b1": b1, "b2": b2,
        "t1": (sum(b1[0]), sum(b1[1])),
        "t2": (sum(b2[0]), sum(b2[1])),
    }

    key = hashlib.sha256(
        e1.tobytes() + e2.tobytes()
        + str(plan["t1"] + plan["t2"]).encode() + stop_after.encode() + ttr_mode.encode() + b"v4"
    ).hexdigest()
    if key in _CACHE:
        nc = _CACHE[key]
    else:
        nc = _build_program(plan, stop_after=stop_after, ttr_mode=ttr_mode)
        _CACHE[key] = nc

    sqc = np.float32(np.sqrt(C))
    common = {
        "x_src": x_src,
        "w1t": np.ascontiguousarray(np.asarray(W1_w, np.float32).T),
        "w2t": np.ascontiguousarray(np.asarray(W2_w, np.float32).T),
        "qwt": np.ascontiguousarray(np.asarray(q_w, np.float32).T),
        "kw_s": np.ascontiguousarray(np.asarray(k_w, np.float32) / sqc),
        "vwt": np.ascontiguousarray(np.asarray(v_w, np.float32).T),
        "b12": (np.asarray(W1_b, np.float32)
                + np.asarray(W2_b, np.float32))[None, :],
        "qb_c": np.asarray(q_b, np.float32)[:, None].copy(),
        "kb_s": (np.asarray(k_b, np.float32) / sqc)[:, None].copy(),
        "vb": np.asarray(v_b, np.float32)[None, :],
        "ident": np.eye(C, dtype=np.float32),
        "identb": np.eye(C, dtype=BFNP),
        "iota_b": np.tile(np.arange(128, dtype=BFNP)[None, :], (128, 1)),
    }
    in_maps = []
    for r in range(NC):
        xm_p = np.zeros((PAD_N, C), np.float32)
        xm_p[perm1[r]] = x_mid[r * PER:(r + 1) * PER]
        xd_p = np.zeros((PAD_N, C), np.float32)
        xd_p[perm2[r]] = x_dst[r * PER:(r + 1) * PER]
        deg1 = np.bincount(m1[own1 == r] - r * PER, minlength=PER)
        deg_slot = np.zeros(PAD_N, np.float32)
        deg_slot[perm1[r]] = deg1
        st1 = _streams(per1_g[r], per1_t[r], per1_lo[r], b1[0], b1[1])
        st2 = _streams(per2_g[r], per2_t[r], per2_lo[r], b2[0], b2[1])
        m = {
            "xmT": np.ascontiguousarray(xm_p.T),
            "xdT": np.ascontiguousarray(xd_p.T),
            "deg_c": np.ascontiguousarray(deg_slot.reshape(NTILE, 128).T),
            **common,
        }
        for sname, (g16, ib) in zip(("lo", "hi"), st1):
            if len(g16):
                m[f"e1g_{sname}"] = _wrap16(g16)
                m[f"e1b_{sname}"] = _tokens(ib)
        for sname, (g16, ib) in zip(("lo", "hi"), st2):
            if len(g16):
                m[f"e2g_{sname}"] = _wrap16(g16)
                m[f"e2b_{sname}"] = _tokens(ib)
        in_maps.append(m)

    _LAST_NC, _LAST_INMAPS, _LAST_PLAN = nc, in_maps, plan

    import os
    if os.environ.get("GNN_SIM"):
        from concourse.bass_interp import MultiCoreSim
        sim = MultiCoreSim(nc, NC)
        for r in range(NC):
            for name, arr in in_maps[r].items():
                sim.cores[r].tensor(name)[:] = arr
        sim.simulate()
        outs = [np.asarray(sim.cores[r].mem_tensor("out")) for r in range(NC)]
    else:
        res = run_bass_kernel_spmd(nc, in_maps, list(range(NC)))
        outs = [res.results[r]["out"] for r in range(NC)]

    out = np.empty((N_NODES, C), np.float32)
    for r in range(NC):
        out[r * PER:(r + 1) * PER] = outs[r][perm2[r]]
    return out
